# revision 1
# baseline (speedup 1.0000x reference)
"""DeepseekV3 MLA attention kernel for 8 trn2 NeuronCores (self-contained).

Sequence-parallel sharding with SPMD-symmetric causal extent classes:
core c owns query rows {128*(e-1)+16c+r : e=1..16, r=0..15}; every core runs
an identical program (one 16-row query sub-block per causal extent class),
with all core-dependence in the data. Latent KV is AllGather'd (bf16, both
orientations). Attention is computed in transposed orientation
(scores^T = latentT^T @ qfoldT) with deferred RMS scales, exp without
max-subtraction, a ones-row matmul for softmax denominators, and 1/L applied
via DRAM-bounce partition-broadcast.
"""

import concourse.mybir as mybir


def split_multiwaits(nc):
    n_split = 0
    for f in nc.m.functions:
        for b in f.blocks:
            insts = b.instructions
            i = 0
            while i < len(insts):
                inst = insts[i]
                si = inst.sync_info
                cap = 2 if inst.opcode == "EventSemaphore" else 1
                if si is not None and si.on_wait and len(si.on_wait) > cap:
                    waits = list(si.on_wait)
                    extra, keep = waits[:-cap], waits[-cap:]
                    for w in extra:
                        nop = mybir.InstNoOp(
                            name=f"{inst.name}_wsplit{n_split}",
                            engine=inst.engine,
                            sync_info=mybir.SyncInfo(on_wait=[w], on_update=[]),
                            bass_nofuse=True,
                            ins=[], outs=[],
                        )
                        n_split += 1
                        insts.insert(i, nop)
                        i += 1
                    inst.sync_info = mybir.SyncInfo(
                        on_wait=keep, on_update=list(si.on_update or []))
                i += 1
    return n_split


import numpy as np
import ml_dtypes
import concourse.bass as bass
import concourse.mybir as mybir
import concourse.tile as tile

F32 = mybir.dt.float32
F32R = mybir.dt.float32r
BF16 = mybir.dt.bfloat16
AF = mybir.ActivationFunctionType

T, HID, H = 2048, 4096, 32
QL, KVL, DN, DR, DV = 1536, 512, 128, 64, 128
RT = KVL + DR            # 576
NC8 = 8                  # cores
TC = 256                 # local t rows per core
NCLS, SB = 16, 16        # extent classes, q sub-block rows
HG = 16                  # heads per group
NG = H // HG             # 2 groups
SCALE = (DN + DR) ** -0.5
EPS = 1e-6
AG_ELEMS = TC * KVL + 5 * 128 * TC   # c_norm + latentT shard (rope tile padded to 128p)

BF16NP = ml_dtypes.bfloat16


def build(debug: bool = False, reps: int = 1, sim_mode: bool = False):
    nc = bass.Bass(num_devices=NC8)

    # ---- I/O ----
    hT_c = nc.dram_tensor("hT_c", (HID, TC), F32R, kind="ExternalInput")
    w_down = nc.dram_tensor("w_down", (HID, QL + RT), F32R, kind="ExternalInput")
    w_qb_nope = nc.dram_tensor("w_qb_nope", (QL, H * DN), BF16, kind="ExternalInput")
    w_qb_rope = nc.dram_tensor("w_qb_rope", (QL, H * DR), BF16, kind="ExternalInput")
    w_ukT = nc.dram_tensor("w_ukT", (H, DN, KVL), BF16, kind="ExternalInput")
    w_uvT = nc.dram_tensor("w_uvT", (H, KVL, DV), BF16, kind="ExternalInput")
    w_o = nc.dram_tensor("w_o", (H * DV, HID), BF16, kind="ExternalInput")
    cosE = nc.dram_tensor("cosE", (TC, DR), F32, kind="ExternalInput")
    sinE = nc.dram_tensor("sinE", (TC, DR), F32, kind="ExternalInput")
    dmask = nc.dram_tensor("dmask", (128, SB), BF16, kind="ExternalInput")
    ident = nc.dram_tensor("ident", (128, 128), F32, kind="ExternalInput")

    out_c = nc.dram_tensor("out_c", (TC, HID), F32, kind="ExternalOutput")
    dbg = {}
    if debug:
        dbg["qcT"] = nc.dram_tensor("dbg_qcT", (QL, TC), BF16, kind="ExternalOutput")
        dbg["lat"] = nc.dram_tensor("dbg_lat", (NC8 * AG_ELEMS,), F32, kind="ExternalOutput")
        dbg["qlT"] = nc.dram_tensor("dbg_qlT", (KVL, HG * TC), BF16, kind="ExternalOutput")
        dbg["qropeT"] = nc.dram_tensor("dbg_qropeT", (DR, HG * TC), BF16, kind="ExternalOutput")
        dbg["L"] = nc.dram_tensor("dbg_L", (1, HG * TC), F32, kind="ExternalOutput")
        dbg["ovT"] = nc.dram_tensor("dbg_ovT", (H * DV, TC), BF16, kind="ExternalOutput")

    rl_dram = nc.dram_tensor("rl_dram", (NG, NCLS // 2, HG * 2 * SB), F32, kind="Internal")
    cc_in = nc.dram_tensor("cc_in", (AG_ELEMS,), BF16, kind="Internal")
    cc_out = nc.dram_tensor("cc_out", (NC8 * AG_ELEMS,), BF16, kind="Internal",
                            addr_space="Shared")
    cc_in_ap = cc_in.ap()
    cc_out_ap = cc_out.ap()

    with tile.TileContext(nc) as tc:
        # ------- persistent pools -------
        persist = tc.alloc_tile_pool(name="persist", bufs=1)

        ident_sb = persist.tile([128, 128], F32, name="ident_sb")
        nc.sync.dma_start(out=ident_sb, in_=ident[:, :])
        ident_bf = persist.tile([128, 128], BF16, name="ident_bf")
        nc.vector.tensor_copy(out=ident_bf, in_=ident_sb)
        mask_sb = persist.tile([128, SB], BF16, name="mask_sb")
        nc.sync.dma_start(out=mask_sb, in_=dmask[:, :])
        ones_sb = persist.tile([128, 1], BF16, name="ones_sb")
        nc.vector.memset(ones_sb, 1.0)
        mask2_sb = persist.tile([128, 2, SB], BF16, name="mask2_sb")
        nc.vector.tensor_copy(out=mask2_sb[:, 0, :], in_=mask_sb)
        nc.vector.memset(mask2_sb[:, 1, :], 1.0)
        eps_sb = persist.tile([128, 1], F32, name="eps_sb")
        nc.vector.memset(eps_sb, EPS)
        cos_sb = persist.tile([128, 2, DR], F32, name="cos_sb")
        sin_sb = persist.tile([128, 2, DR], F32, name="sin_sb")
        cosv = cosE.ap().rearrange("(m p) d -> m p d", m=2)
        sinv = sinE.ap().rearrange("(m p) d -> m p d", m=2)
        for m in range(2):
            nc.sync.dma_start(out=cos_sb[:, m, :], in_=cosv[m])
            nc.sync.dma_start(out=sin_sb[:, m, :], in_=sinv[m])

        # ================= reps (device-timing: run body N times) ========
        rep_pools = []
        for rep in range(reps):
          prep = tc.alloc_tile_pool(name=f"rep{rep}", bufs=1)
          rep_pools.append(prep)
          # q_cT persists into phase 2
          q_cT = prep.tile([128, 12, TC], BF16, name=f"q_cT{rep}")  # [qk][128, 256] bf16

          # ================= Phase 1: down-proj =================
          with tc.tile_pool(name="p1", bufs=1) as p1, \
               tc.tile_pool(name="p1s", bufs=4) as p1s, \
               tc.tile_pool(name="p1ps", bufs=3, space="PSUM") as p1ps:
              # resident hT (4.2MB)
              hT_sb = p1.tile([128, 32, TC], F32R, name="hT_sb")  # [k][128, 256]
              hTv = hT_c.ap().rearrange("(k p) t -> k p t", k=32)
              for k in range(32):
                  nc.sync.dma_start(out=hT_sb[:, k, :], in_=hTv[k])

              q_c = p1.tile([128, 2, QL], F32, name="q_c")      # [m][128, 1536]
              latent = p1.tile([128, 2, KVL], F32, name="latent")
              krope = p1.tile([128, 2, DR], F32, name="krope")

              # chunks of w_down columns: 3x512 (q_c), 1x512 (latent), 1x64 (krope)
              wdv = w_down.ap().rearrange("(k p) n -> k p n", k=32)
              for ch in range(5):
                  c0 = ch * 512
                  cw = 512 if ch < 4 else 64
                  pss = []
                  for m in range(2):
                      ps = p1ps.tile([128, 512], F32, tag=f"dps{m}", bufs=2)
                      pss.append(ps)
                  for k in range(32):
                      wt = p1s.tile([128, 512], F32R, tag="wdt")
                      nc.sync.dma_start(out=wt[:, :cw], in_=wdv[k][:, c0:c0 + cw])
                      for m in range(2):
                          nc.tensor.matmul(
                              pss[m][:, :cw],
                              hT_sb[:, k, bass.ts(m, 128)],
                              wt[:, :cw],
                              start=(k == 0), stop=(k == 31))
                  for m in range(2):
                      if ch < 3:
                          nc.scalar.copy(out=q_c[:, m, c0:c0 + 512], in_=pss[m][:, :512])
                      elif ch == 3:
                          nc.scalar.copy(out=latent[:, m, :], in_=pss[m][:, :512])
                      else:
                          nc.scalar.copy(out=krope[:, m, :], in_=pss[m][:, :64])

              # ---- norms (t-orientation, deferred scales) ----
              stat = p1s.tile([128, 3, 6], F32, tag="stat")
              mv = p1s.tile([128, 2], F32, tag="mv")
              for m in range(2):
                  # q-side
                  sq = p1s.tile([128, QL], F32, tag="sq")
                  nc.vector.tensor_mul(sq, q_c[:, m, :], q_c[:, m, :])
                  for sg in range(3):
                      nc.vector.bn_stats(out=stat[:, sg, :], in_=sq[:, bass.ts(sg, 512)])
                  nc.vector.bn_aggr(out=mv, in_=stat)
                  rstd = p1s.tile([128, 1], F32, tag="rstd")
                  nc.scalar.activation(out=rstd, in_=mv[:, 0:1], func=AF.Sqrt, bias=eps_sb)
                  nc.vector.reciprocal(out=rstd, in_=rstd)
                  nc.scalar.mul(out=rstd, in_=rstd, mul=SCALE)
                  nc.vector.tensor_scalar_mul(out=q_c[:, m, :], in0=q_c[:, m, :], scalar1=rstd)
                  # kv-side
                  sq2 = p1s.tile([128, KVL], F32, tag="sq2")
                  nc.vector.tensor_mul(sq2, latent[:, m, :], latent[:, m, :])
                  nc.vector.bn_stats(out=stat[:, 0, :], in_=sq2)
                  nc.vector.bn_aggr(out=mv, in_=stat[:, 0, :])
                  rstd2 = p1s.tile([128, 1], F32, tag="rstd2")
                  nc.scalar.activation(out=rstd2, in_=mv[:, 0:1], func=AF.Sqrt, bias=eps_sb)
                  nc.vector.reciprocal(out=rstd2, in_=rstd2)
                  nc.vector.tensor_scalar_mul(out=latent[:, m, :], in0=latent[:, m, :], scalar1=rstd2)
                  # k-rope rotate
                  kr3 = krope[:, m, :].rearrange("p (a two) -> p a two", two=2)
                  sw = p1s.tile([128, DR], F32, tag="swk")
                  sw3 = sw.rearrange("p (a two) -> p a two", two=2)
                  nc.vector.tensor_copy(out=sw3[:, :, 0], in_=kr3[:, :, 1])
                  nc.vector.tensor_copy(out=sw3[:, :, 1], in_=kr3[:, :, 0])
                  nc.vector.tensor_mul(krope[:, m, :], krope[:, m, :], cos_sb[:, m, :])
                  tmp = p1s.tile([128, DR], F32, tag="tmpk")
                  nc.vector.tensor_mul(tmp, sw, sin_sb[:, m, :])
                  nc.vector.tensor_add(krope[:, m, :], krope[:, m, :], tmp)

              # ---- transposes ----
              # q_cT
              for m in range(2):
                  for qk in range(12):
                      tp = p1ps.tile([128, 128], F32, tag="tps", bufs=2)
                      nc.tensor.transpose(tp, q_c[:, m, bass.ts(qk, 128)], ident_sb)
                      nc.vector.tensor_copy(out=q_cT[:, qk, bass.ts(m, 128)], in_=tp)
              # latent shard bf16: latT_sh [5][<=128, 256], c_norm bf16 [2][128, 512]
              latT_sh = p1.tile([128, 5, TC], BF16, name="latT_sh")
              nc.vector.memset(latT_sh[64:, 4, :], 0.0)
              cn_bf = p1.tile([128, 2, KVL], BF16, name="cn_bf")
              for m in range(2):
                  nc.vector.tensor_copy(out=cn_bf[:, m, :], in_=latent[:, m, :])
                  for rk in range(4):
                      tp = p1ps.tile([128, 128], F32, tag="tps", bufs=2)
                      nc.tensor.transpose(tp, latent[:, m, bass.ts(rk, 128)], ident_sb)
                      nc.vector.tensor_copy(out=latT_sh[:, rk, bass.ts(m, 128)], in_=tp)
                  tp = p1ps.tile([128, 128], F32, tag="tps", bufs=2)
                  nc.tensor.transpose(tp[:64, :], krope[:, m, :], ident_sb)
                  nc.vector.tensor_copy(out=latT_sh[:64, 4, bass.ts(m, 128)], in_=tp[:64, :])

              # ---- ship shard to cc_in, AllGather ----
              cnv = cc_in_ap[0:TC * KVL].rearrange("(p m r) -> p m r", p=128, m=2)
              nc.sync.dma_start(out=cnv, in_=cn_bf)   # local-row-major [256, 512]
              # wait: local rows are (m, p)? cn_bf is [p, m, r]; local row = m*128+p.
              # Use explicit per-m views to keep row-major order in DRAM:
              ltv = cc_in_ap[TC * KVL:].rearrange("(p k t) -> p k t", p=128, k=5)
              nc.sync.dma_start(out=ltv, in_=latT_sh)
              if sim_mode:
                  for cpy in range(NC8):
                      nc.sync.dma_start(
                          out=cc_out_ap[cpy * AG_ELEMS:(cpy + 1) * AG_ELEMS].rearrange("(p f) -> p f", p=128),
                          in_=cc_in_ap[:].rearrange("(p f) -> p f", p=128))
              else:
                  nc.gpsimd.collective_compute(
                      "AllGather", mybir.AluOpType.bypass,
                      replica_groups=[list(range(NC8))],
                      ins=[cc_in_ap[:]], outs=[cc_out_ap[:]],
                  )

          # ================= load gathered K/V =================
          # latT [5][128, 2048 cols = (cls 16, core 8, 16)] bf16 ; C [16][128(cls rows (c,r)), 512] bf16
          latT = prep.tile([128, 5, 2048], BF16, name=f"latT{rep}")
          C_sb = prep.tile([128, 16, KVL], BF16, name=f"C_sb{rep}")
          # C: one DMA per s-tile j gathering all 8 chunks.
          # chunk c local row l=16j+r lives at flat off c*AG + l*KVL (cn region,
          # row-major (p, m, r) = l-major since l=(m*128+p)... actually (p,m,r)
          # layout: flat = p*2*KVL + m*KVL + r with l = m*128+p.
          cnv_all = cc_out_ap[:].rearrange("(c e) -> c e", c=NC8)
          for j in range(NCLS):
              m, p0 = divmod(SB * j, 128)  # rows 16j..16j+15 are in part m, p=p0..p0+15
              csrc = bass.AP(tensor=cc_out_ap.tensor, offset=p0 * 2 * KVL + m * KVL,
                             ap=[[AG_ELEMS, NC8], [2 * KVL, SB], [1, KVL]])
              nc.sync.dma_start(out=C_sb[:, j, :], in_=csrc)
          for c in range(NC8):
              chunk = cc_out_ap[c * AG_ELEMS:(c + 1) * AG_ELEMS]
              lt = chunk[TC * KVL:].rearrange("(p k t) -> p k t", p=128, k=5)
              for m in range(2):
                  lsrc = lt[:, :, bass.ts(m, 128)].rearrange("p k (j r) -> p k j r", j=8)
                  dst = latT[:, :, :].rearrange("p k (j c r) -> p k j c r", j=16, c=8)
                  for kk in range(5):
                      nc.sync.dma_start(out=dst[:, kk, m * 8:(m + 1) * 8, c, :],
                                        in_=lsrc[:, kk, :, :])

          if debug:
              lt_dbg = dbg["lat"].ap().rearrange("(c e) -> c e", c=NC8)
              with tc.tile_pool(name="dbglat", bufs=2) as pdl:
                  for c in range(NC8):
                      tmpd = pdl.tile([128, AG_ELEMS // 128], BF16, tag="tmpd")
                      nc.sync.dma_start(
                          out=tmpd,
                          in_=cc_out_ap[c * AG_ELEMS:(c + 1) * AG_ELEMS].rearrange("(p f) -> p f", p=128))
                      tmpf = pdl.tile([128, AG_ELEMS // 128], F32, tag="tmpf")
                      nc.vector.tensor_copy(out=tmpf, in_=tmpd)
                      nc.sync.dma_start(
                          out=lt_dbg[c].rearrange("(p f) -> p f", p=128), in_=tmpf)


          # ================= Phase 2: per head-group =================
          ovT_sb = prep.tile([128, H, TC], BF16, name=f"ovT_sb{rep}")
          p3s = tc.alloc_tile_pool(name="p3s", bufs=8)
          wov = w_o.ap().rearrange("(k p) n -> k p n", k=32)
          wot_tiles = {}
          for n in range(8):
              for k in range(32):
                  wot = p3s.tile([128, 512], BF16, tag="wot", name=f"wot{rep}_{n}_{k}")
                  nc.sync.dma_start(out=wot, in_=wov[k][:, bass.ts(n, 512)])
                  wot_tiles[(n, k)] = wot
          for g in range(NG):
              with tc.tile_pool(name=f"g{g}", bufs=1) as pg, \
                   tc.tile_pool(name=f"g{g}s", bufs=4) as pgs, \
                   tc.tile_pool(name=f"g{g}s2", bufs=2) as pgs2:
                  qlT = pg.tile([128, 4, HG, TC], BF16, name=f"qlT{g}")
                  qropeT = pg.tile([64, HG, TC], BF16, name=f"qropeT{g}")
                  up_ctx = tc.tile_pool(name=f"g{g}ps", bufs=2, space="PSUM")
                  pgps = up_ctx.__enter__()

                  # ---- q up-proj nope + fold per head ----
                  wqbv = w_qb_nope.ap().rearrange("(k p) n -> k p n", k=12)
                  for h4 in range(HG // 4):
                      h0 = g * HG + h4 * 4
                      qn_pss = []
                      for i in range(4):
                          qnp = pgps.tile([128, TC], F32, tag=f"qn_ps{i}", name=f"qn{g}_{h4}_{i}", bufs=1)
                          qn_pss.append(qnp)
                      for qk in range(12):
                          wt = pgs.tile([128, 4, 128], BF16, tag="wqbt")
                          nc.sync.dma_start(
                              out=wt, in_=wqbv[qk][:, h0 * 128:(h0 + 4) * 128].rearrange(
                                  "p (i d) -> p i d", i=4))
                          for i in range(4):
                              nc.tensor.matmul(qn_pss[i], wt[:, i, :], q_cT[:, qk, :],
                                               start=(qk == 0), stop=(qk == 11))
                      for i in range(4):
                          hl = h4 * 4 + i
                          h = g * HG + hl
                          qn_sb = pgs.tile([128, TC], BF16, tag="qn_sb")
                          nc.scalar.copy(out=qn_sb, in_=qn_pss[i])
                          wuk = pgs2.tile([128, KVL], BF16, tag="wuk")
                          nc.sync.dma_start(out=wuk, in_=w_ukT.ap()[h])
                          for rk in range(4):
                              fps = pgps.tile([128, TC], F32, tag="fold_ps", bufs=2)
                              nc.tensor.matmul(fps, wuk[:, bass.ts(rk, 128)],
                                               qn_sb, start=True, stop=True)
                              nc.vector.tensor_copy(out=qlT[:, rk, hl, :], in_=fps)

                  # ---- q rope for group ----
                  wqrv = w_qb_rope.ap().rearrange("(k p) n -> k p n", k=12)
                  qr = pg.tile([128, 2, HG * DR], F32, name=f"qr{g}")
                  for m in range(2):
                      for nt in range(2):
                          ps = pgps.tile([128, 512], F32, tag="qr_ps", bufs=1)
                          for qk in range(12):
                              wt = pgs2.tile([128, 512], BF16, tag="wqrt")
                              nc.sync.dma_start(
                                  out=wt, in_=wqrv[qk][:, g * HG * DR + nt * 512:g * HG * DR + nt * 512 + 512])
                              nc.tensor.matmul(ps, q_cT[:, qk, bass.ts(m, 128)],
                                               wt, start=(qk == 0), stop=(qk == 11))
                          nc.scalar.copy(out=qr[:, m, bass.ts(nt, 512)], in_=ps)
                      # rotate
                      qr4 = qr[:, m, :].rearrange("p (h a two) -> p h a two", h=HG, two=2)
                      sw = pgs2.tile([128, HG * DR], F32, tag="swq")
                      sw4 = sw.rearrange("p (h a two) -> p h a two", h=HG, two=2)
                      nc.vector.tensor_copy(out=sw4[:, :, :, 0], in_=qr4[:, :, :, 1])
                      nc.vector.tensor_copy(out=sw4[:, :, :, 1], in_=qr4[:, :, :, 0])
                      qr3 = qr[:, m, :].rearrange("p (h d) -> p h d", h=HG)
                      sw3 = sw.rearrange("p (h d) -> p h d", h=HG)
                      cos_bc = cos_sb[:, m, :].unsqueeze(1).broadcast_to([128, HG, DR])
                      sin_bc = sin_sb[:, m, :].unsqueeze(1).broadcast_to([128, HG, DR])
                      nc.vector.tensor_mul(qr3, qr3, cos_bc)
                      nc.vector.tensor_mul(sw3, sw3, sin_bc)
                      qrb = pgs2.tile([128, HG * DR], BF16, tag="qrb")
                      nc.vector.tensor_add(qrb.rearrange("p (h d) -> p h d", h=HG), qr3, sw3)
                      # transpose per head: [128, 64] -> [64, 128] via PE
                      qrb3 = qrb.rearrange("p (h d) -> p h d", h=HG)
                      for hl in range(HG):
                          tp = pgps.tile([64, 128], BF16, tag="rtp", bufs=1)
                          nc.tensor.transpose(tp, qrb3[:, hl, :], ident_bf)
                          nc.vector.tensor_copy(out=qropeT[:, hl, bass.ts(m, 128)], in_=tp)

                  if debug and g == 0:
                      for rk in range(4):
                          dv = dbg["qlT"].ap()[bass.ts(rk, 128), :].rearrange("p (h t) -> p h t", h=HG)
                          nc.sync.dma_start(out=dv, in_=qlT[:, rk, :, :])
                      nc.sync.dma_start(
                          out=dbg["qropeT"].ap().rearrange("p (h t) -> p h t", h=HG)[:, :, :],
                          in_=qropeT)

                  up_ctx.__exit__(None, None, None)

                  # ---- attention ----
                  olT = pg.tile([128, 4, HG, TC], BF16, name=f"olT{g}")
                  rl_all = pg.tile([128, HG, TC], F32, name=f"rl{g}")
                  at_ctx = tc.tile_pool(name=f"g{g}aps", bufs=2, space="PSUM")
                  pgps = at_ctx.__enter__()
                  po_ctx = tc.tile_pool(name=f"g{g}po", bufs=1, space="PSUM")
                  pgpo = po_ctx.__enter__()
                  for pr in range(NCLS // 2):
                      e1, e2 = 2 * pr + 1, 2 * pr + 2
                      qs2 = slice(SB * 2 * pr, SB * (2 * pr + 2))     # 32 q-cols
                      ol_ps = [pgpo.tile([128, HG * 2 * SB], F32, tag=f"ol{rk}", name=f"ol{rk}_{g}_{pr}")
                               for rk in range(4)]
                      l_ps = pgpo.tile([1, HG * 2 * SB], F32, tag="lps", name=f"lps_{g}_{pr}")
                      # joint s-tiles cover classes e1 AND e2 (N=512)
                      for s in range(e1):
                          sc_ps = pgps.tile([128, HG * 2 * SB], F32, tag="sc_ps")
                          for rk in range(5):
                              if rk < 4:
                                  lhs = latT[:, rk, bass.ts(s, 128)]
                                  rhs = qlT[:, rk, :, qs2]
                              else:
                                  lhs = latT[:64, 4, bass.ts(s, 128)]
                                  rhs = qropeT[:, :, qs2]
                              nc.tensor.matmul(sc_ps, lhs, rhs, start=(rk == 0), stop=(rk == 4))
                          pt = pgs.tile([128, HG * 2 * SB], BF16, tag="pt")
                          nc.scalar.activation(out=pt, in_=sc_ps, func=AF.Exp)
                          if s == e1 - 1:
                              # class e1's diagonal: mask its half, ones on e2 half
                              ptm = pt.rearrange("p (h q) -> p h q", h=HG)
                              m2 = bass.AP(tensor=mask2_sb.tensor, offset=mask2_sb.offset,
                                           ap=list(mask2_sb.ap[:1]) + [[0, HG], [1, 2 * SB]])
                              nc.vector.tensor_mul(ptm, ptm, m2)
                          for rk in range(4):
                              nc.tensor.matmul(ol_ps[rk], C_sb[:, s, bass.ts(rk, 128)], pt,
                                               start=(s == 0), stop=False,
                                               skip_group_check=True)
                          nc.tensor.matmul(l_ps, ones_sb, pt,
                                           start=(s == 0), stop=False,
                                           skip_group_check=True)
                      # solo s-tile for class e2 (N=256), diagonal-masked
                      s = e2 - 1
                      qe2 = slice(SB * (2 * pr + 1), SB * (2 * pr + 2))
                      sc_ps = pgps.tile([128, HG * 2 * SB], F32, tag="sc_ps")
                      for rk in range(5):
                          if rk < 4:
                              lhs = latT[:, rk, bass.ts(s, 128)]
                              rhs = qlT[:, rk, :, qe2]
                          else:
                              lhs = latT[:64, 4, bass.ts(s, 128)]
                              rhs = qropeT[:, :, qe2]
                          nc.tensor.matmul(sc_ps[:, :HG * SB], lhs, rhs,
                                           start=(rk == 0), stop=(rk == 4))
                      pt = pgs.tile([128, HG * 2 * SB], BF16, tag="pt")
                      pt3 = pt[:, :HG * SB].rearrange("p (h q) -> p h q", h=HG)
                      nc.scalar.activation(out=pt[:, :HG * SB], in_=sc_ps[:, :HG * SB], func=AF.Exp)
                      mask_bc = mask_sb.unsqueeze(1).broadcast_to([128, HG, SB])
                      nc.vector.tensor_mul(pt3, pt3, mask_bc)
                      olv = [op.rearrange("p (h t q) -> p h t q", h=HG, t=2) for op in ol_ps]
                      for rk in range(4):
                          nc.tensor.matmul(olv[rk][:, :, 1, :], C_sb[:, s, bass.ts(rk, 128)],
                                           pt3, start=False, stop=True,
                                           skip_group_check=True)
                      lv = l_ps.rearrange("p (h t q) -> p h t q", h=HG, t=2)
                      nc.tensor.matmul(lv[:, :, 1, :], ones_sb, pt3,
                                       start=False, stop=True, skip_group_check=True)
                      # note: e1 columns got stop via... close their group with the solo MM only
                      for rk in range(4):
                          nc.vector.tensor_copy(
                              out=olT[:, rk, :, qs2],
                              in_=ol_ps[rk].rearrange("p (h q) -> p h q", h=HG))
                      l_row = pgs.tile([1, HG * 2 * SB], F32, tag="l_row")
                      nc.vector.reciprocal(out=l_row, in_=l_ps)
                      nc.sync.dma_start(out=rl_dram.ap()[g, pr], in_=l_row)
                      bsrc = rl_dram.ap()[g, pr]
                      bc_ap = bass.AP(tensor=bsrc.tensor, offset=bsrc.offset,
                                      ap=[[0, 128], [2 * SB, HG], [1, 2 * SB]])
                      nc.sync.dma_start(out=rl_all[:, :, qs2], in_=bc_ap)

                  if debug and g == 0:
                      nc.sync.dma_start(
                          out=dbg["L"].ap().rearrange("o (h t) -> o h t", h=HG),
                          in_=rl_all[0:1, :, :])

                  po_ctx.__exit__(None, None, None)
                  at_ctx.__exit__(None, None, None)
                  uv_ctx = tc.tile_pool(name=f"g{g}ups", bufs=2, space="PSUM")
                  pgps = uv_ctx.__enter__()
                  # ---- UV (1/L already broadcast in rl_all) ----
                  for hl in range(HG):
                      h = g * HG + hl
                      wuv = pgs.tile([128, 4, DV], BF16, tag="wuv")
                      nc.sync.dma_start(
                          out=wuv, in_=w_uvT.ap()[h].rearrange("(rk p) d -> p rk d", rk=4))
                      uv_ps = pgps.tile([128, TC], F32, tag="uv_ps")
                      for rk in range(4):
                          nc.tensor.matmul(uv_ps, wuv[:, rk, :], olT[:, rk, hl, :],
                                           start=(rk == 0), stop=(rk == 3))
                      nc.vector.tensor_mul(ovT_sb[:, h, :], uv_ps, rl_all[:, hl, :])
                  if True:
                      uv_ctx.__exit__(None, None, None)

          if debug:
              for qk in range(12):
                  nc.sync.dma_start(out=dbg["qcT"].ap()[bass.ts(qk, 128), :], in_=q_cT[:, qk, :])

          # ================= Phase 3: o_proj =================
          with tc.tile_pool(name="p3o", bufs=4) as p3o, \
               tc.tile_pool(name="p3ps", bufs=2, space="PSUM") as p3ps:
              if debug:
                  nc.sync.dma_start(
                      out=dbg["ovT"].ap().rearrange("(h p) t -> p h t", h=H),
                      in_=ovT_sb)
              ocv = out_c.ap().rearrange("(m p) n -> m p n", m=2)
              for n in range(8):
                  pss = []
                  for m in range(2):
                      ps = p3ps.tile([128, 512], F32, tag=f"o_ps{m}")
                      pss.append(ps)
                  for k in range(32):
                      for m in range(2):
                          nc.tensor.matmul(pss[m], ovT_sb[:, k, bass.ts(m, 128)],
                                           wot_tiles[(n, k)], start=(k == 0), stop=(k == 31))
                  for m in range(2):
                      ob = p3o.tile([128, 512], F32, tag="ob")
                      nc.scalar.copy(out=ob, in_=pss[m])
                      nc.sync.dma_start(out=ocv[m][:, bass.ts(n, 512)], in_=ob)
          p3s.release()
          prep.release()

        persist.release()
    return nc


# ======================= host side =======================

def core_rows(c):
    return np.concatenate([np.arange(128 * e + SB * c, 128 * e + SB * c + SB)
                           for e in range(NCLS)])


def host_prep(inputs):
    hs = np.asarray(inputs["hidden_states"], np.float32)
    pos = np.asarray(inputs["positions"], np.int32)
    w_qa = np.asarray(inputs["w_qa"], np.float32)
    qa_s = np.asarray(inputs["qa_ln_scale"], np.float32)
    w_qb = np.asarray(inputs["w_qb"], np.float32) * qa_s[:, None]
    w_kva = np.asarray(inputs["w_kva"], np.float32)
    kva_s = np.asarray(inputs["kva_ln_scale"], np.float32)
    w_uk = np.asarray(inputs["w_uk"], np.float32) * kva_s[:, None, None]
    w_uv = np.asarray(inputs["w_uv"], np.float32) * kva_s[:, None, None]
    w_o = np.ascontiguousarray(np.asarray(inputs["w_o"], np.float32)).astype(BF16NP)

    w_down = np.ascontiguousarray(np.concatenate([w_qa, w_kva], axis=1))
    w_qb3 = w_qb.reshape(QL, H, DN + DR)
    w_qb_nope = np.ascontiguousarray(w_qb3[:, :, :DN].reshape(QL, -1)).astype(BF16NP)
    w_qb_rope = np.ascontiguousarray(w_qb3[:, :, DN:].reshape(QL, -1)).astype(BF16NP)
    w_ukT = np.ascontiguousarray(w_uk.transpose(1, 2, 0)).astype(BF16NP)  # [H, DN, KVL]
    w_uvT = np.ascontiguousarray(w_uv.transpose(1, 0, 2)).astype(BF16NP)  # [H, KVL, DV]

    inv_freq = 10000.0 ** (-np.arange(0, DR, 2, dtype=np.float32) / DR)
    ang = pos.astype(np.float32)[:, None] * inv_freq[None, :]
    cosE = np.repeat(np.cos(ang), 2, axis=1).astype(np.float32)
    sinE = np.repeat(np.sin(ang), 2, axis=1).astype(np.float32)
    sinE[:, 0::2] *= -1.0

    hT = hs.T
    ident = np.eye(128, dtype=np.float32)
    shared = dict(w_down=w_down, w_qb_nope=w_qb_nope, w_qb_rope=w_qb_rope,
                  w_ukT=w_ukT, w_uvT=w_uvT, w_o=w_o, ident=ident)
    per_core = []
    for c in range(NC8):
        rows = core_rows(c)
        mask = (np.arange(128)[:, None] <= (SB * c + np.arange(SB))[None, :]).astype(BF16NP)
        per_core.append(dict(
            hT_c=np.ascontiguousarray(hT[:, rows]),
            cosE=np.ascontiguousarray(cosE[rows]),
            sinE=np.ascontiguousarray(sinE[rows]),
            dmask=mask,
        ))
    return shared, per_core


def make_in_maps(inputs):
    shared, per_core = host_prep(inputs)
    return [dict(shared, **pc) for pc in per_core]


def unshard(results):
    out = np.zeros((T, HID), np.float32)
    for c in range(NC8):
        out[core_rows(c)] = results[c]["out_c"]
    return out


_NC_CACHE = {}


def kernel(**inputs):
    from concourse.bass_utils import run_bass_kernel_spmd
    if "nc" not in _NC_CACHE:
        nc = build(debug=False)
        split_multiwaits(nc)
        _NC_CACHE["nc"] = nc
    nc = _NC_CACHE["nc"]
    in_maps = make_in_maps(inputs)
    res = run_bass_kernel_spmd(nc, in_maps, core_ids=list(range(NC8)))
    return unshard(res.results)



# revision 9
# speedup vs baseline: 1.1081x; 1.1081x over previous
"""DeepseekV3 MLA attention kernel for 8 trn2 NeuronCores (self-contained).

Sequence-parallel sharding with SPMD-symmetric causal extent classes:
core c owns query rows {128*(e-1)+16c+r : e=1..16, r=0..15}; every core runs
an identical program (one 16-row query sub-block per causal extent class),
with all core-dependence in the data. Latent KV is AllGather'd (bf16, both
orientations). Attention is computed in transposed orientation
(scores^T = latentT^T @ qfoldT) with deferred RMS scales, exp without
max-subtraction, a ones-row matmul for softmax denominators, and 1/L applied
via DRAM-bounce partition-broadcast.
"""

import concourse.mybir as mybir


def split_multiwaits(nc):
    n_split = 0
    for f in nc.m.functions:
        for b in f.blocks:
            insts = b.instructions
            i = 0
            while i < len(insts):
                inst = insts[i]
                si = inst.sync_info
                cap = 2 if inst.opcode == "EventSemaphore" else 1
                if si is not None and si.on_wait and len(si.on_wait) > cap:
                    waits = list(si.on_wait)
                    extra, keep = waits[:-cap], waits[-cap:]
                    for w in extra:
                        nop = mybir.InstNoOp(
                            name=f"{inst.name}_wsplit{n_split}",
                            engine=inst.engine,
                            sync_info=mybir.SyncInfo(on_wait=[w], on_update=[]),
                            bass_nofuse=True,
                            ins=[], outs=[],
                        )
                        n_split += 1
                        insts.insert(i, nop)
                        i += 1
                    inst.sync_info = mybir.SyncInfo(
                        on_wait=keep, on_update=list(si.on_update or []))
                i += 1
    return n_split


import numpy as np
import ml_dtypes
import concourse.bass as bass
import concourse.mybir as mybir
import concourse.tile as tile

F32 = mybir.dt.float32
F32R = mybir.dt.float32r
BF16 = mybir.dt.bfloat16
AF = mybir.ActivationFunctionType

T, HID, H = 2048, 4096, 32
QL, KVL, DN, DR, DV = 1536, 512, 128, 64, 128
RT = KVL + DR            # 576
NC8 = 8                  # cores
TC = 256                 # local t rows per core
NCLS, SB = 16, 16        # extent classes, q sub-block rows
HG = 16                  # heads per group
NG = H // HG             # 2 groups
SCALE = (DN + DR) ** -0.5
EPS = 1e-6
AG_ELEMS = TC * KVL + 5 * 128 * TC   # c_norm + latentT shard (rope tile padded to 128p)

BF16NP = ml_dtypes.bfloat16


def build(debug: bool = False, reps: int = 1, sim_mode: bool = False):
    nc = bass.Bass(num_devices=NC8)

    # ---- I/O ----
    hT_c = nc.dram_tensor("hT_c", (HID, TC), BF16, kind="ExternalInput")
    w_down = nc.dram_tensor("w_down", (HID, QL + RT), BF16, kind="ExternalInput")
    w_qb_nope = nc.dram_tensor("w_qb_nope", (QL, H * DN), BF16, kind="ExternalInput")
    w_qb_rope = nc.dram_tensor("w_qb_rope", (QL, H * DR), BF16, kind="ExternalInput")
    w_ukT = nc.dram_tensor("w_ukT", (H, DN, KVL), BF16, kind="ExternalInput")
    w_uvT = nc.dram_tensor("w_uvT", (H, KVL, DV), BF16, kind="ExternalInput")
    w_o = nc.dram_tensor("w_o", (8, 32, 128, 512), BF16, kind="ExternalInput")
    cosE = nc.dram_tensor("cosE", (TC, DR), F32, kind="ExternalInput")
    sinE = nc.dram_tensor("sinE", (TC, DR), F32, kind="ExternalInput")
    dmask = nc.dram_tensor("dmask", (128, SB), BF16, kind="ExternalInput")
    ident = nc.dram_tensor("ident", (128, 128), F32, kind="ExternalInput")

    out_c = nc.dram_tensor("out_c", (TC, HID), F32, kind="ExternalOutput")
    dbg = {}
    if debug:
        dbg["qcT"] = nc.dram_tensor("dbg_qcT", (QL, TC), BF16, kind="ExternalOutput")
        dbg["lat"] = nc.dram_tensor("dbg_lat", (NC8 * AG_ELEMS,), F32, kind="ExternalOutput")
        dbg["qlT"] = nc.dram_tensor("dbg_qlT", (KVL, HG * TC), BF16, kind="ExternalOutput")
        dbg["qropeT"] = nc.dram_tensor("dbg_qropeT", (DR, HG * TC), BF16, kind="ExternalOutput")
        dbg["L"] = nc.dram_tensor("dbg_L", (1, HG * TC), F32, kind="ExternalOutput")
        dbg["ovT"] = nc.dram_tensor("dbg_ovT", (H * DV, TC), BF16, kind="ExternalOutput")

    rl_dram = nc.dram_tensor("rl_dram", (NG, NCLS // 2, HG * 2 * SB), F32, kind="Internal")
    cc_in = nc.dram_tensor("cc_in", (AG_ELEMS,), BF16, kind="Internal")
    cc_out = nc.dram_tensor("cc_out", (NC8 * AG_ELEMS,), BF16, kind="Internal",
                            addr_space="Shared")
    cc_in_ap = cc_in.ap()
    cc_out_ap = cc_out.ap()

    with tile.TileContext(nc) as tc:
        # ------- persistent pools -------
        persist = tc.alloc_tile_pool(name="persist", bufs=1)

        ident_sb = persist.tile([128, 128], F32, name="ident_sb")
        nc.sync.dma_start(out=ident_sb, in_=ident[:, :])
        ident_bf = persist.tile([128, 128], BF16, name="ident_bf")
        nc.vector.tensor_copy(out=ident_bf, in_=ident_sb)
        mask_sb = persist.tile([128, SB], BF16, name="mask_sb")
        nc.sync.dma_start(out=mask_sb, in_=dmask[:, :])
        ones_sb = persist.tile([128, 1], BF16, name="ones_sb")
        nc.vector.memset(ones_sb, 1.0)
        mask2_sb = persist.tile([128, 2, SB], BF16, name="mask2_sb")
        nc.vector.tensor_copy(out=mask2_sb[:, 0, :], in_=mask_sb)
        nc.vector.memset(mask2_sb[:, 1, :], 1.0)
        eps_sb = persist.tile([128, 1], F32, name="eps_sb")
        nc.vector.memset(eps_sb, EPS)
        cos_sb = persist.tile([128, 2, DR], F32, name="cos_sb")
        sin_sb = persist.tile([128, 2, DR], F32, name="sin_sb")
        cosv = cosE.ap().rearrange("(m p) d -> m p d", m=2)
        sinv = sinE.ap().rearrange("(m p) d -> m p d", m=2)
        for m in range(2):
            nc.sync.dma_start(out=cos_sb[:, m, :], in_=cosv[m])
            nc.sync.dma_start(out=sin_sb[:, m, :], in_=sinv[m])

        # ================= reps (device-timing: run body N times) ========
        rep_pools = []
        for rep in range(reps):
          prep = tc.alloc_tile_pool(name=f"rep{rep}", bufs=1)
          rep_pools.append(prep)
          # q_cT persists into phase 2
          q_cT = prep.tile([128, 12, TC], BF16, name=f"q_cT{rep}")  # [qk][128, 256] bf16

          # ================= Phase 1: down-proj =================
          with tc.tile_pool(name="p1", bufs=1) as p1, \
               tc.tile_pool(name="p1s", bufs=4) as p1s, \
               tc.tile_pool(name="p1ps", bufs=3, space="PSUM") as p1ps:
              # resident hT (2.1MB bf16)
              hT_sb = p1.tile([128, 32, TC], BF16, name="hT_sb")  # [k][128, 256]
              hTv = hT_c.ap().rearrange("(k p) t -> k p t", k=32)
              for k8 in range(4):
                  nc.sync.dma_start(out=hT_sb[:, 8 * k8:8 * k8 + 8, :],
                                    in_=hTv[8 * k8:8 * k8 + 8].rearrange("k p t -> p k t"))

              q_c = p1.tile([128, 2, QL], F32, name="q_c")      # [m][128, 1536]
              latent = p1.tile([128, 2, KVL], F32, name="latent")
              krope = p1.tile([128, 2, DR], F32, name="krope")

              # chunks of w_down columns: 3x512 (q_c), 1x512 (latent), 1x64 (krope)
              wdv = w_down.ap().rearrange("(k p) n -> k p n", k=32)
              for ch in range(5):
                  c0 = ch * 512
                  cw = 512 if ch < 4 else 64
                  pss = []
                  for m in range(2):
                      ps = p1ps.tile([128, 512], F32, tag=f"dps{m}", bufs=2)
                      pss.append(ps)
                  for k in range(32):
                      wt = p1s.tile([128, 512], BF16, tag="wdt")
                      nc.sync.dma_start(out=wt[:, :cw], in_=wdv[k][:, c0:c0 + cw])
                      for m in range(2):
                          nc.tensor.matmul(
                              pss[m][:, :cw],
                              hT_sb[:, k, bass.ts(m, 128)],
                              wt[:, :cw],
                              start=(k == 0), stop=(k == 31))
                  for m in range(2):
                      if ch < 3:
                          nc.scalar.copy(out=q_c[:, m, c0:c0 + 512], in_=pss[m][:, :512])
                      elif ch == 3:
                          nc.scalar.copy(out=latent[:, m, :], in_=pss[m][:, :512])
                      else:
                          nc.scalar.copy(out=krope[:, m, :], in_=pss[m][:, :64])

              # ---- norms (t-orientation, deferred scales) ----
              stat = p1s.tile([128, 3, 6], F32, tag="stat")
              mv = p1s.tile([128, 2], F32, tag="mv")
              for m in range(2):
                  # q-side
                  sq = p1s.tile([128, QL], F32, tag="sq")
                  nc.vector.tensor_mul(sq, q_c[:, m, :], q_c[:, m, :])
                  for sg in range(3):
                      nc.vector.bn_stats(out=stat[:, sg, :], in_=sq[:, bass.ts(sg, 512)])
                  nc.vector.bn_aggr(out=mv, in_=stat)
                  rstd = p1s.tile([128, 1], F32, tag="rstd")
                  nc.scalar.activation(out=rstd, in_=mv[:, 0:1], func=AF.Sqrt, bias=eps_sb)
                  nc.vector.reciprocal(out=rstd, in_=rstd)
                  nc.scalar.mul(out=rstd, in_=rstd, mul=SCALE)
                  nc.vector.tensor_scalar_mul(out=q_c[:, m, :], in0=q_c[:, m, :], scalar1=rstd)
                  # kv-side
                  sq2 = p1s.tile([128, KVL], F32, tag="sq2")
                  nc.vector.tensor_mul(sq2, latent[:, m, :], latent[:, m, :])
                  nc.vector.bn_stats(out=stat[:, 0, :], in_=sq2)
                  nc.vector.bn_aggr(out=mv, in_=stat[:, 0, :])
                  rstd2 = p1s.tile([128, 1], F32, tag="rstd2")
                  nc.scalar.activation(out=rstd2, in_=mv[:, 0:1], func=AF.Sqrt, bias=eps_sb)
                  nc.vector.reciprocal(out=rstd2, in_=rstd2)
                  nc.vector.tensor_scalar_mul(out=latent[:, m, :], in0=latent[:, m, :], scalar1=rstd2)
                  # k-rope rotate
                  kr3 = krope[:, m, :].rearrange("p (a two) -> p a two", two=2)
                  sw = p1s.tile([128, DR], F32, tag="swk")
                  sw3 = sw.rearrange("p (a two) -> p a two", two=2)
                  nc.vector.tensor_copy(out=sw3[:, :, 0], in_=kr3[:, :, 1])
                  nc.vector.tensor_copy(out=sw3[:, :, 1], in_=kr3[:, :, 0])
                  nc.vector.tensor_mul(krope[:, m, :], krope[:, m, :], cos_sb[:, m, :])
                  tmp = p1s.tile([128, DR], F32, tag="tmpk")
                  nc.vector.tensor_mul(tmp, sw, sin_sb[:, m, :])
                  nc.vector.tensor_add(krope[:, m, :], krope[:, m, :], tmp)

              # ---- transposes ----
              # q_cT
              for m in range(2):
                  for qk in range(12):
                      tp = p1ps.tile([128, 128], F32, tag="tps", bufs=2)
                      nc.tensor.transpose(tp, q_c[:, m, bass.ts(qk, 128)], ident_sb)
                      nc.vector.tensor_copy(out=q_cT[:, qk, bass.ts(m, 128)], in_=tp)
              # latent shard bf16: latT_sh [5][<=128, 256], c_norm bf16 [2][128, 512]
              latT_sh = p1.tile([128, 5, TC], BF16, name="latT_sh")
              nc.vector.memset(latT_sh[64:, 4, :], 0.0)
              cn_bf = p1.tile([128, 2, KVL], BF16, name="cn_bf")
              for m in range(2):
                  nc.vector.tensor_copy(out=cn_bf[:, m, :], in_=latent[:, m, :])
                  for rk in range(4):
                      tp = p1ps.tile([128, 128], F32, tag="tps", bufs=2)
                      nc.tensor.transpose(tp, latent[:, m, bass.ts(rk, 128)], ident_sb)
                      nc.vector.tensor_copy(out=latT_sh[:, rk, bass.ts(m, 128)], in_=tp)
                  tp = p1ps.tile([128, 128], F32, tag="tps", bufs=2)
                  nc.tensor.transpose(tp[:64, :], krope[:, m, :], ident_sb)
                  nc.vector.tensor_copy(out=latT_sh[:64, 4, bass.ts(m, 128)], in_=tp[:64, :])

              # ---- ship shard to cc_in, AllGather ----
              cnv = cc_in_ap[0:TC * KVL].rearrange("(p m r) -> p m r", p=128, m=2)
              nc.sync.dma_start(out=cnv, in_=cn_bf)   # local-row-major [256, 512]
              # wait: local rows are (m, p)? cn_bf is [p, m, r]; local row = m*128+p.
              # Use explicit per-m views to keep row-major order in DRAM:
              ltv = cc_in_ap[TC * KVL:].rearrange("(p k t) -> p k t", p=128, k=5)
              nc.sync.dma_start(out=ltv, in_=latT_sh)
              if sim_mode:
                  for cpy in range(NC8):
                      nc.sync.dma_start(
                          out=cc_out_ap[cpy * AG_ELEMS:(cpy + 1) * AG_ELEMS].rearrange("(p f) -> p f", p=128),
                          in_=cc_in_ap[:].rearrange("(p f) -> p f", p=128))
              else:
                  nc.gpsimd.collective_compute(
                      "AllGather", mybir.AluOpType.bypass,
                      replica_groups=[list(range(NC8))],
                      ins=[cc_in_ap[:]], outs=[cc_out_ap[:]],
                  )

          # ================= load gathered K/V =================
          # latT [5][128, 2048 cols = (cls 16, core 8, 16)] bf16 ; C [16][128(cls rows (c,r)), 512] bf16
          latT = prep.tile([128, 5, 2048], BF16, name=f"latT{rep}")
          C_sb = prep.tile([128, 16, KVL], BF16, name=f"C_sb{rep}")
          # C: one DMA per s-tile j gathering all 8 chunks.
          # chunk c local row l=16j+r lives at flat off c*AG + l*KVL (cn region,
          # row-major (p, m, r) = l-major since l=(m*128+p)... actually (p,m,r)
          # layout: flat = p*2*KVL + m*KVL + r with l = m*128+p.
          cnv_all = cc_out_ap[:].rearrange("(c e) -> c e", c=NC8)
          for j in range(NCLS):
              m, p0 = divmod(SB * j, 128)  # rows 16j..16j+15 are in part m, p=p0..p0+15
              csrc = bass.AP(tensor=cc_out_ap.tensor, offset=p0 * 2 * KVL + m * KVL,
                             ap=[[AG_ELEMS, NC8], [2 * KVL, SB], [1, KVL]])
              nc.sync.dma_start(out=C_sb[:, j, :], in_=csrc)
          for c in range(NC8):
              chunk = cc_out_ap[c * AG_ELEMS:(c + 1) * AG_ELEMS]
              lt = chunk[TC * KVL:].rearrange("(p k t) -> p k t", p=128, k=5)
              for m in range(2):
                  lsrc = lt[:, :, bass.ts(m, 128)].rearrange("p k (j r) -> p k j r", j=8)
                  dst = latT[:, :, :].rearrange("p k (j c r) -> p k j c r", j=16, c=8)
                  for kk in range(5):
                      nc.sync.dma_start(out=dst[:, kk, m * 8:(m + 1) * 8, c, :],
                                        in_=lsrc[:, kk, :, :])

          if debug:
              lt_dbg = dbg["lat"].ap().rearrange("(c e) -> c e", c=NC8)
              with tc.tile_pool(name="dbglat", bufs=2) as pdl:
                  for c in range(NC8):
                      tmpd = pdl.tile([128, AG_ELEMS // 128], BF16, tag="tmpd")
                      nc.sync.dma_start(
                          out=tmpd,
                          in_=cc_out_ap[c * AG_ELEMS:(c + 1) * AG_ELEMS].rearrange("(p f) -> p f", p=128))
                      tmpf = pdl.tile([128, AG_ELEMS // 128], F32, tag="tmpf")
                      nc.vector.tensor_copy(out=tmpf, in_=tmpd)
                      nc.sync.dma_start(
                          out=lt_dbg[c].rearrange("(p f) -> p f", p=128), in_=tmpf)


          # ================= Phase 2: per head-group =================
          ovT_sb = prep.tile([128, H, TC], BF16, name=f"ovT_sb{rep}")
          p3s = tc.alloc_tile_pool(name="p3s", bufs=3)
          wot_tiles = {}
          for n in range(8):
              for k4 in range(8):
                  wot = p3s.tile([128, 4, 512], BF16, tag="wot", name=f"wot{rep}_{n}_{k4}")
                  nc.sync.dma_start(
                      out=wot, in_=w_o.ap()[n, 4 * k4:4 * k4 + 4].rearrange("k p n -> p k n"))
                  for kk in range(4):
                      wot_tiles[(n, 4 * k4 + kk)] = wot[:, kk, :]
          for g in range(NG):
              with tc.tile_pool(name=f"g{g}", bufs=1) as pg, \
                   tc.tile_pool(name=f"g{g}s", bufs=4) as pgs, \
                   tc.tile_pool(name=f"g{g}s2", bufs=2) as pgs2:
                  qlT = pg.tile([128, 4, HG, TC], BF16, name=f"qlT{g}")
                  qropeT = pg.tile([64, HG, TC], BF16, name=f"qropeT{g}")
                  up_ctx = tc.tile_pool(name=f"g{g}ps", bufs=2, space="PSUM")
                  pgps = up_ctx.__enter__()

                  # ---- q up-proj nope + fold per head ----
                  wqbv = w_qb_nope.ap().rearrange("(k p) n -> k p n", k=12)
                  for h4 in range(HG // 4):
                      h0 = g * HG + h4 * 4
                      qn_pss = []
                      for i in range(4):
                          qnp = pgps.tile([128, TC], F32, tag=f"qn_ps{i}", name=f"qn{g}_{h4}_{i}", bufs=1)
                          qn_pss.append(qnp)
                      for qk in range(12):
                          wt = pgs.tile([128, 4, 128], BF16, tag="wqbt")
                          nc.sync.dma_start(
                              out=wt, in_=wqbv[qk][:, h0 * 128:(h0 + 4) * 128].rearrange(
                                  "p (i d) -> p i d", i=4))
                          for i in range(4):
                              nc.tensor.matmul(qn_pss[i], wt[:, i, :], q_cT[:, qk, :],
                                               start=(qk == 0), stop=(qk == 11))
                      for i in range(4):
                          hl = h4 * 4 + i
                          h = g * HG + hl
                          qn_sb = pgs.tile([128, TC], BF16, tag="qn_sb")
                          nc.scalar.copy(out=qn_sb, in_=qn_pss[i])
                          wuk = pgs2.tile([128, KVL], BF16, tag="wuk")
                          nc.sync.dma_start(out=wuk, in_=w_ukT.ap()[h])
                          for rk in range(4):
                              fps = pgps.tile([128, TC], F32, tag="fold_ps", bufs=2)
                              nc.tensor.matmul(fps, wuk[:, bass.ts(rk, 128)],
                                               qn_sb, start=True, stop=True)
                              nc.vector.tensor_copy(out=qlT[:, rk, hl, :], in_=fps)

                  # ---- q rope for group ----
                  wqrv = w_qb_rope.ap().rearrange("(k p) n -> k p n", k=12)
                  qr = pg.tile([128, 2, HG * DR], F32, name=f"qr{g}")
                  for m in range(2):
                      for nt in range(2):
                          ps = pgps.tile([128, 512], F32, tag="qr_ps", bufs=1)
                          for qk in range(12):
                              wt = pgs2.tile([128, 512], BF16, tag="wqrt")
                              nc.sync.dma_start(
                                  out=wt, in_=wqrv[qk][:, g * HG * DR + nt * 512:g * HG * DR + nt * 512 + 512])
                              nc.tensor.matmul(ps, q_cT[:, qk, bass.ts(m, 128)],
                                               wt, start=(qk == 0), stop=(qk == 11))
                          nc.scalar.copy(out=qr[:, m, bass.ts(nt, 512)], in_=ps)
                      # rotate
                      qr4 = qr[:, m, :].rearrange("p (h a two) -> p h a two", h=HG, two=2)
                      sw = pgs2.tile([128, HG * DR], F32, tag="swq")
                      sw4 = sw.rearrange("p (h a two) -> p h a two", h=HG, two=2)
                      nc.vector.tensor_copy(out=sw4[:, :, :, 0], in_=qr4[:, :, :, 1])
                      nc.vector.tensor_copy(out=sw4[:, :, :, 1], in_=qr4[:, :, :, 0])
                      qr3 = qr[:, m, :].rearrange("p (h d) -> p h d", h=HG)
                      sw3 = sw.rearrange("p (h d) -> p h d", h=HG)
                      cos_bc = cos_sb[:, m, :].unsqueeze(1).broadcast_to([128, HG, DR])
                      sin_bc = sin_sb[:, m, :].unsqueeze(1).broadcast_to([128, HG, DR])
                      nc.vector.tensor_mul(qr3, qr3, cos_bc)
                      nc.vector.tensor_mul(sw3, sw3, sin_bc)
                      qrb = pgs2.tile([128, HG * DR], BF16, tag="qrb")
                      nc.vector.tensor_add(qrb.rearrange("p (h d) -> p h d", h=HG), qr3, sw3)
                      # transpose per head: [128, 64] -> [64, 128] via PE
                      qrb3 = qrb.rearrange("p (h d) -> p h d", h=HG)
                      for hl in range(HG):
                          tp = pgps.tile([64, 128], BF16, tag="rtp", bufs=1)
                          nc.tensor.transpose(tp, qrb3[:, hl, :], ident_bf)
                          nc.vector.tensor_copy(out=qropeT[:, hl, bass.ts(m, 128)], in_=tp)

                  if debug and g == 0:
                      for rk in range(4):
                          dv = dbg["qlT"].ap()[bass.ts(rk, 128), :].rearrange("p (h t) -> p h t", h=HG)
                          nc.sync.dma_start(out=dv, in_=qlT[:, rk, :, :])
                      nc.sync.dma_start(
                          out=dbg["qropeT"].ap().rearrange("p (h t) -> p h t", h=HG)[:, :, :],
                          in_=qropeT)

                  up_ctx.__exit__(None, None, None)

                  # ---- attention ----
                  olT = pg.tile([128, 4, HG, TC], BF16, name=f"olT{g}")
                  rl_all = pg.tile([128, HG, TC], F32, name=f"rl{g}")
                  at_ctx = tc.tile_pool(name=f"g{g}aps", bufs=2, space="PSUM")
                  pgps = at_ctx.__enter__()
                  po_ctx = tc.tile_pool(name=f"g{g}po", bufs=1, space="PSUM")
                  pgpo = po_ctx.__enter__()
                  for pr in range(NCLS // 2):
                      e1, e2 = 2 * pr + 1, 2 * pr + 2
                      qs2 = slice(SB * 2 * pr, SB * (2 * pr + 2))     # 32 q-cols
                      ol_ps = [pgpo.tile([128, HG * 2 * SB], F32, tag=f"ol{rk}", name=f"ol{rk}_{g}_{pr}")
                               for rk in range(4)]
                      l_ps = pgpo.tile([1, HG * 2 * SB], F32, tag="lps", name=f"lps_{g}_{pr}")
                      # joint s-tiles cover classes e1 AND e2 (N=512)
                      for s in range(e1):
                          sc_ps = pgps.tile([128, HG * 2 * SB], F32, tag="sc_ps")
                          for rk in range(5):
                              if rk < 4:
                                  lhs = latT[:, rk, bass.ts(s, 128)]
                                  rhs = qlT[:, rk, :, qs2]
                              else:
                                  lhs = latT[:64, 4, bass.ts(s, 128)]
                                  rhs = qropeT[:, :, qs2]
                              nc.tensor.matmul(sc_ps, lhs, rhs, start=(rk == 0), stop=(rk == 4))
                          pt = pgs.tile([128, HG * 2 * SB], BF16, tag="pt")
                          nc.scalar.activation(out=pt, in_=sc_ps, func=AF.Exp)
                          if s == e1 - 1:
                              # class e1's diagonal: mask its half, ones on e2 half
                              ptm = pt.rearrange("p (h q) -> p h q", h=HG)
                              m2 = bass.AP(tensor=mask2_sb.tensor, offset=mask2_sb.offset,
                                           ap=list(mask2_sb.ap[:1]) + [[0, HG], [1, 2 * SB]])
                              nc.vector.tensor_mul(ptm, ptm, m2)
                          for rk in range(4):
                              nc.tensor.matmul(ol_ps[rk], C_sb[:, s, bass.ts(rk, 128)], pt,
                                               start=(s == 0), stop=False,
                                               skip_group_check=True)
                          nc.tensor.matmul(l_ps, ones_sb, pt,
                                           start=(s == 0), stop=False,
                                           skip_group_check=True)
                      # solo s-tile for class e2 (N=256), diagonal-masked
                      s = e2 - 1
                      qe2 = slice(SB * (2 * pr + 1), SB * (2 * pr + 2))
                      sc_ps = pgps.tile([128, HG * 2 * SB], F32, tag="sc_ps")
                      for rk in range(5):
                          if rk < 4:
                              lhs = latT[:, rk, bass.ts(s, 128)]
                              rhs = qlT[:, rk, :, qe2]
                          else:
                              lhs = latT[:64, 4, bass.ts(s, 128)]
                              rhs = qropeT[:, :, qe2]
                          nc.tensor.matmul(sc_ps[:, :HG * SB], lhs, rhs,
                                           start=(rk == 0), stop=(rk == 4))
                      pt = pgs.tile([128, HG * 2 * SB], BF16, tag="pt")
                      pt3 = pt[:, :HG * SB].rearrange("p (h q) -> p h q", h=HG)
                      nc.scalar.activation(out=pt[:, :HG * SB], in_=sc_ps[:, :HG * SB], func=AF.Exp)
                      mask_bc = mask_sb.unsqueeze(1).broadcast_to([128, HG, SB])
                      nc.vector.tensor_mul(pt3, pt3, mask_bc)
                      olv = [op.rearrange("p (h t q) -> p h t q", h=HG, t=2) for op in ol_ps]
                      for rk in range(4):
                          nc.tensor.matmul(olv[rk][:, :, 1, :], C_sb[:, s, bass.ts(rk, 128)],
                                           pt3, start=False, stop=True,
                                           skip_group_check=True)
                      lv = l_ps.rearrange("p (h t q) -> p h t q", h=HG, t=2)
                      nc.tensor.matmul(lv[:, :, 1, :], ones_sb, pt3,
                                       start=False, stop=True, skip_group_check=True)
                      # note: e1 columns got stop via... close their group with the solo MM only
                      for rk in range(4):
                          nc.vector.tensor_copy(
                              out=olT[:, rk, :, qs2],
                              in_=ol_ps[rk].rearrange("p (h q) -> p h q", h=HG))
                      l_row = pgs.tile([1, HG * 2 * SB], F32, tag="l_row")
                      nc.vector.reciprocal(out=l_row, in_=l_ps)
                      nc.sync.dma_start(out=rl_dram.ap()[g, pr], in_=l_row)
                      bsrc = rl_dram.ap()[g, pr]
                      bc_ap = bass.AP(tensor=bsrc.tensor, offset=bsrc.offset,
                                      ap=[[0, 128], [2 * SB, HG], [1, 2 * SB]])
                      nc.sync.dma_start(out=rl_all[:, :, qs2], in_=bc_ap)

                  if debug and g == 0:
                      nc.sync.dma_start(
                          out=dbg["L"].ap().rearrange("o (h t) -> o h t", h=HG),
                          in_=rl_all[0:1, :, :])

                  po_ctx.__exit__(None, None, None)
                  at_ctx.__exit__(None, None, None)
                  uv_ctx = tc.tile_pool(name=f"g{g}ups", bufs=2, space="PSUM")
                  pgps = uv_ctx.__enter__()
                  # ---- UV (1/L already broadcast in rl_all) ----
                  for hl in range(HG):
                      h = g * HG + hl
                      wuv = pgs.tile([128, 4, DV], BF16, tag="wuv")
                      nc.sync.dma_start(
                          out=wuv, in_=w_uvT.ap()[h].rearrange("(rk p) d -> p rk d", rk=4))
                      uv_ps = pgps.tile([128, TC], F32, tag="uv_ps")
                      for rk in range(4):
                          nc.tensor.matmul(uv_ps, wuv[:, rk, :], olT[:, rk, hl, :],
                                           start=(rk == 0), stop=(rk == 3))
                      nc.vector.tensor_mul(ovT_sb[:, h, :], uv_ps, rl_all[:, hl, :])
                  if True:
                      uv_ctx.__exit__(None, None, None)

          if debug:
              for qk in range(12):
                  nc.sync.dma_start(out=dbg["qcT"].ap()[bass.ts(qk, 128), :], in_=q_cT[:, qk, :])

          # ================= Phase 3: o_proj =================
          with tc.tile_pool(name="p3o", bufs=4) as p3o, \
               tc.tile_pool(name="p3ps", bufs=2, space="PSUM") as p3ps:
              if debug:
                  nc.sync.dma_start(
                      out=dbg["ovT"].ap().rearrange("(h p) t -> p h t", h=H),
                      in_=ovT_sb)
              ocv = out_c.ap().rearrange("(m p) n -> m p n", m=2)
              for n in range(8):
                  pss = []
                  for m in range(2):
                      ps = p3ps.tile([128, 512], F32, tag=f"o_ps{m}")
                      pss.append(ps)
                  for k in range(32):
                      for m in range(2):
                          nc.tensor.matmul(pss[m], ovT_sb[:, k, bass.ts(m, 128)],
                                           wot_tiles[(n, k)], start=(k == 0), stop=(k == 31))
                  for m in range(2):
                      ob = p3o.tile([128, 512], F32, tag="ob")
                      nc.scalar.copy(out=ob, in_=pss[m])
                      nc.sync.dma_start(out=ocv[m][:, bass.ts(n, 512)], in_=ob)
          p3s.release()
          prep.release()

        persist.release()
    return nc


# ======================= host side =======================

def core_rows(c):
    return np.concatenate([np.arange(128 * e + SB * c, 128 * e + SB * c + SB)
                           for e in range(NCLS)])


def host_prep(inputs):
    hs = np.asarray(inputs["hidden_states"], np.float32)
    pos = np.asarray(inputs["positions"], np.int32)
    w_qa = np.asarray(inputs["w_qa"], np.float32)
    qa_s = np.asarray(inputs["qa_ln_scale"], np.float32)
    w_qb = np.asarray(inputs["w_qb"], np.float32) * qa_s[:, None]
    w_kva = np.asarray(inputs["w_kva"], np.float32)
    kva_s = np.asarray(inputs["kva_ln_scale"], np.float32)
    w_uk = np.asarray(inputs["w_uk"], np.float32) * kva_s[:, None, None]
    w_uv = np.asarray(inputs["w_uv"], np.float32) * kva_s[:, None, None]
    w_o = np.ascontiguousarray(
        np.asarray(inputs["w_o"], np.float32).reshape(32, 128, 8, 512)
        .transpose(2, 0, 1, 3)).astype(BF16NP)

    w_down = np.ascontiguousarray(
        np.concatenate([w_qa, w_kva], axis=1)).astype(BF16NP)
    w_qb3 = w_qb.reshape(QL, H, DN + DR)
    w_qb_nope = np.ascontiguousarray(w_qb3[:, :, :DN].reshape(QL, -1)).astype(BF16NP)
    w_qb_rope = np.ascontiguousarray(w_qb3[:, :, DN:].reshape(QL, -1)).astype(BF16NP)
    w_ukT = np.ascontiguousarray(w_uk.transpose(1, 2, 0)).astype(BF16NP)  # [H, DN, KVL]
    w_uvT = np.ascontiguousarray(w_uv.transpose(1, 0, 2)).astype(BF16NP)  # [H, KVL, DV]

    inv_freq = 10000.0 ** (-np.arange(0, DR, 2, dtype=np.float32) / DR)
    ang = pos.astype(np.float32)[:, None] * inv_freq[None, :]
    cosE = np.repeat(np.cos(ang), 2, axis=1).astype(np.float32)
    sinE = np.repeat(np.sin(ang), 2, axis=1).astype(np.float32)
    sinE[:, 0::2] *= -1.0

    hT = hs.T.astype(BF16NP)
    ident = np.eye(128, dtype=np.float32)
    shared = dict(w_down=w_down, w_qb_nope=w_qb_nope, w_qb_rope=w_qb_rope,
                  w_ukT=w_ukT, w_uvT=w_uvT, w_o=w_o, ident=ident)
    per_core = []
    for c in range(NC8):
        rows = core_rows(c)
        mask = (np.arange(128)[:, None] <= (SB * c + np.arange(SB))[None, :]).astype(BF16NP)
        per_core.append(dict(
            hT_c=np.ascontiguousarray(hT[:, rows]),
            cosE=np.ascontiguousarray(cosE[rows]),
            sinE=np.ascontiguousarray(sinE[rows]),
            dmask=mask,
        ))
    return shared, per_core


def make_in_maps(inputs):
    shared, per_core = host_prep(inputs)
    return [dict(shared, **pc) for pc in per_core]


def unshard(results):
    out = np.zeros((T, HID), np.float32)
    for c in range(NC8):
        out[core_rows(c)] = results[c]["out_c"]
    return out


_NC_CACHE = {}


def kernel(**inputs):
    from concourse.bass_utils import run_bass_kernel_spmd
    if "nc" not in _NC_CACHE:
        nc = build(debug=False)
        split_multiwaits(nc)
        _NC_CACHE["nc"] = nc
    nc = _NC_CACHE["nc"]
    in_maps = make_in_maps(inputs)
    res = run_bass_kernel_spmd(nc, in_maps, core_ids=list(range(NC8)))
    return unshard(res.results)

